# revision 1
# baseline (speedup 1.0000x reference)
"""GCN (5-layer) + global mean pool + MLP head on 8 trn2 NeuronCores.

Strategy (dest-sharded, AllGather):
  - Factorize GCN norm: with dis = rsqrt(deg), y = dis * (h @ W), the layer is
      h'[v] = relu(dis[v] * (sum_{u->v} y[u] + y[v]) + b)
    so message passing is a pure gather+sum of 64B rows.
  - Nodes are sharded across 8 cores (12500 each). Each core gathers its
    in-edges' source rows from a DRAM y-table via dma_gather (int16 indices,
    64B elements from a 256B-strided table), reduces slot-groups on DVE,
    applies the epilogue, computes its y' slice, and AllGathers y' each layer.
  - The y-table is split in 4 windows of 25600 rows so indices fit int16;
    per (core, source-window) destination orderings are degree-sorted to
    minimize slot padding, then realigned with 3 small index gathers.
"""
import inspect
import re

import numpy as np

import concourse.bass as bass
import concourse.bacc as bacc
import concourse.tile as tile
import concourse.mybir as mybir
from concourse.bass2jax import run_bass_via_pjrt
from concourse.masks import make_identity

F32 = mybir.dt.float32
I16 = mybir.dt.int16
AL = mybir.AluOpType

N_NODES = 100000
N_EDGES = 3200000
N_GRAPHS = 1000
HID = 16
C = 8                    # cores
NPC = N_NODES // C       # 12500 nodes per core
P = 128
TILES = 98               # ceil(12500/128)
NPAD = TILES * P         # 12544
WIN = 25600              # table window rows (int16-addressable)
PAD_LOCAL = 2 * NPAD     # zero rows at window-local [25088, 25600)
TBL_ROWS = 4 * WIN       # 102400
ROW = 64                 # table row stride in f32 (256B)
GPC = N_GRAPHS // C      # 125 graphs per core
MAXCOL = 32              # max token columns per gather instruction (4096 idx)
RRELU_SLOPE = (1.0 / 8.0 + 1.0 / 3.0) / 2.0

# table row base for core c: window c//2, upper/lower half
def _core_base(c):
    return WIN * (c // 2) + NPAD * (c % 2)


def _make_patched_dma_gather():
    """dma_gather with the elem_size%256 assert dropped (the 256B constraint
    is on the row stride; 64B payloads from a 256B-strided table work)."""
    src = inspect.getsource(bass.BassGpSimd.dma_gather)
    src = src.replace(
        "assert (\n            elem_size_bytes > 0 and elem_size_bytes % 256 == 0\n        )  # transpose restriction",
        "assert elem_size_bytes > 0")
    src = re.sub(r"^    def dma_gather", "def dma_gather_patched", src, flags=re.M)
    src = "\n".join(l[4:] if l.startswith("    ") else l for l in src.splitlines())
    ns = dict(bass.__dict__)
    exec(src, ns)
    return ns["dma_gather_patched"]


_dma_gather = _make_patched_dma_gather()


def _wrap_idx(tokens):
    """[128, cols] token array (token i at (i%128, i//128)) -> [128, cols*8]
    int16 wrapped index layout (idx i at (i%16, i//16), replicated x8)."""
    p, cols = tokens.shape
    assert p == P
    flat = tokens.T.reshape(-1)                  # i = col*128 + p
    w16 = flat.reshape(-1, 16).T                 # [16, cols*8]
    return np.tile(w16, (8, 1)).astype(np.int16)


def _preprocess(x, edge_index, batch):
    """Build per-core device inputs + shared (cross-core identical) plan."""
    src = np.asarray(edge_index[0], dtype=np.int64)
    dst = np.asarray(edge_index[1], dtype=np.int64)
    batch = np.asarray(batch, dtype=np.int64)
    x = np.asarray(x, dtype=np.float32)

    deg = np.bincount(dst, minlength=N_NODES).astype(np.float32) + 1.0

    dcore = dst // NPC
    srng = src // WIN  # 0 only for now; real range needs table rows (below)

    # per (core, range) in-degree of each dest; range of an edge = source's
    # table window = (source core)//2
    src_core = src // NPC
    rng = src_core // 2
    key = dst * 4 + rng
    cnt4 = np.bincount(key, minlength=N_NODES * 4).reshape(N_NODES, 4)

    # destination orderings: perm[c][r] = local dest ids in processing order
    perm = [[None] * 4 for _ in range(C)]
    rank_in_perm = [[None] * 4 for _ in range(C)]
    for c in range(C):
        lo = c * NPC
        for r in range(4):
            d = cnt4[lo:lo + NPC, r]
            order = np.argsort(-d, kind="stable")
            perm[c][r] = order
            rk = np.empty(NPC, dtype=np.int64)
            rk[order] = np.arange(NPC)
            rank_in_perm[c][r] = rk

    # canonical (sigma) order per core = perm[c][0]; table row of each node
    row_of_node = np.empty(N_NODES, dtype=np.int64)
    for c in range(C):
        lo = c * NPC
        row_of_node[lo + perm[c][0]] = _core_base(c) + np.arange(NPC)

    # K structure per (range, tile): cross-core max of tile-max degree
    Ks = np.zeros((4, TILES), dtype=np.int64)
    for c in range(C):
        for r in range(4):
            d_sorted = cnt4[c * NPC:(c + 1) * NPC, r][perm[c][r]]
            d_pad = np.concatenate([d_sorted, np.zeros(NPAD - NPC, np.int64)])
            tile_max = d_pad.reshape(TILES, P).max(axis=1)
            Ks[r] = np.maximum(Ks[r], tile_max)
    Ks = np.maximum(Ks, 1)
    assert Ks.max() <= MAXCOL, f"tile K {Ks.max()} exceeds {MAXCOL}"

    # instructions per range: each = MAXCOL token columns (4096 idx), ONE K,
    # T = n tiles, slot-major columns (col = slot*T + tile_j) so reduces and
    # copies are contiguous. entry: (r, instr_idx_in_range, t_start, T, K)
    instrs = []
    tile_base = [np.zeros(TILES, np.int64) for _ in range(4)]  # instr base col
    tile_T = [np.zeros(TILES, np.int64) for _ in range(4)]
    tile_j = [np.zeros(TILES, np.int64) for _ in range(4)]
    n_instr_r = [0] * 4
    for r in range(4):
        ii = 0
        t = 0
        while t < TILES:
            K = int(Ks[r][t])
            cap = MAXCOL // K
            T = 1
            while (T < cap and t + T < TILES and int(Ks[r][t + T]) == K):
                T += 1
            for j in range(T):
                tile_base[r][t + j] = ii * MAXCOL
                tile_T[r][t + j] = T
                tile_j[r][t + j] = j
            instrs.append((r, ii, t, T, K))
            ii += 1
            t += T
        n_instr_r[r] = ii

    # token arrays per core per range: [128, COLS_r] window-local row indices
    tok = [[np.full((P, n_instr_r[r] * MAXCOL), PAD_LOCAL, dtype=np.int64)
            for r in range(4)] for _ in range(C)]
    erank = np.empty(N_EDGES, dtype=np.int64)  # dest rank in perm[dcore][rng]
    for c in range(C):
        m = dcore == c
        for r in range(4):
            mm = m & (rng == r)
            erank[mm] = rank_in_perm[c][r][dst[mm] - c * NPC]
    # slot index of each edge within its (dest, range) group
    order = np.lexsort((erank, rng, dcore))
    so_dcore, so_rng, so_rank = dcore[order], rng[order], erank[order]
    gkey = (so_dcore * 4 + so_rng) * NPC + so_rank
    starts = np.concatenate([[True], gkey[1:] != gkey[:-1]])
    gidx = np.cumsum(starts) - 1
    first = np.flatnonzero(starts)
    slot = np.arange(len(order)) - first[gidx]
    so_locrow = (row_of_node[src[order]]) % WIN
    for c in range(C):
        m = so_dcore == c
        for r in range(4):
            mm = m & (so_rng == r)
            rk = so_rank[mm]
            tl = rk // P
            col = tile_base[r][tl] + slot[mm] * tile_T[r][tl] + tile_j[r][tl]
            tok[c][r][rk % P, col] = so_locrow[mm]

    # align-gather indices (ranges 1..3): position i (canonical) -> rank in
    # perm_r of the dest at canonical position i
    align_idx = [[None] * 4 for _ in range(C)]
    for c in range(C):
        for r in range(1, 4):
            ai = np.arange(NPAD, dtype=np.int64)
            ai[:NPC] = rank_in_perm[c][r][perm[c][0]]
            align_idx[c][r] = ai

    # pooling: graph g -> core g // GPC ; member rows per (core, graph, range)
    g_of_node = batch
    node_rows = row_of_node
    node_rng = node_rows // WIN
    pkey = (g_of_node * 4 + node_rng)
    pcnt = np.bincount(pkey, minlength=N_GRAPHS * 4).reshape(N_GRAPHS, 4)
    Kp = np.zeros(4, dtype=np.int64)
    for r in range(4):
        Kp[r] = max(1, pcnt[:, r].max())
    # chunk pool ranges into <=MAXCOL col chunks (accumulate extra chunks)
    pool_chunks = []  # (r, col_off_in_pool_r, K_chunk, accum)
    pool_cols = []
    for r in range(4):
        off = 0
        K = int(Kp[r])
        acc = False
        while K > 0:
            k = min(K, MAXCOL)
            pool_chunks.append((r, off, k, acc))
            off += MAXCOL
            K -= k
            acc = True
        pool_cols.append(off)

    ptok = [[np.full((P, pool_cols[r]), PAD_LOCAL, dtype=np.int64)
             for r in range(4)] for _ in range(C)]
    porder = np.lexsort((g_of_node, node_rng))
    po_g, po_rng = g_of_node[porder], node_rng[porder]
    pk = po_g * 4 + po_rng
    pstarts = np.concatenate([[True], pk[1:] != pk[:-1]])
    pgidx = np.cumsum(pstarts) - 1
    pfirst = np.flatnonzero(pstarts)
    pslot = np.arange(len(porder)) - pfirst[pgidx]
    po_locrow = node_rows[porder] % WIN
    for r in range(4):
        m = po_rng == r
        g = po_g[m]
        c = g // GPC
        part = g % GPC
        sl = pslot[m]
        col = (sl // MAXCOL) * MAXCOL + (sl % MAXCOL)  # == sl; chunks stride MAXCOL
        for cc in range(C):
            mm = c == cc
            ptok[cc][r][part[mm], col[mm]] = po_locrow[m][mm]

    cnt_graph = np.bincount(batch, minlength=N_GRAPHS).astype(np.float32)
    cnt_graph = np.maximum(cnt_graph, 1.0)

    # per-core host arrays
    per_core = []
    for c in range(C):
        lo = c * NPC
        sigma = perm[c][0]
        nodes_sigma = lo + sigma  # node id at canonical position i
        deg_pad = np.ones(NPAD, dtype=np.float32)
        deg_pad[:NPC] = deg[nodes_sigma]
        deg_tiles = deg_pad.reshape(TILES, P).T.copy()  # [128, 98]

        nblk = (TILES + 7) // 8
        xp = np.zeros((nblk * 8 * P, 4), dtype=np.float32)
        xp[:NPC] = x[nodes_sigma]
        xt = xp.reshape(nblk, 8, P, 4).transpose(1, 3, 0, 2).reshape(32, nblk * P).copy()

        gather_w = np.concatenate(
            [_wrap_idx(tok[c][r]) for r in range(4)], axis=1)
        align_w_parts = []
        for r in range(1, 4):
            a = np.zeros((P, P), dtype=np.int64)
            a[:, :TILES] = align_idx[c][r].reshape(TILES, P).T
            align_w_parts.append(_wrap_idx(a))
        align_w = np.concatenate(align_w_parts, axis=1)
        pool_w = np.concatenate(
            [_wrap_idx(ptok[c][r]) for r in range(4)], axis=1)

        cnt_c = np.ones((P, 1), dtype=np.float32)
        cnt_c[:GPC, 0] = cnt_graph[c * GPC:(c + 1) * GPC]

        per_core.append(dict(
            deg_tiles=deg_tiles, xt=xt, gather_w=gather_w,
            align_w=align_w, pool_w=pool_w, cnt=cnt_c))

    plan = dict(instrs=instrs, Ks=Ks, n_instr_r=n_instr_r,
                pool_chunks=pool_chunks, pool_cols=pool_cols)
    return per_core, plan


def _reduce_equalK(nc, g, T, K, elem=HID, base_off=0):
    """In-place slot reduce of G viewed as [128, T, K, elem] -> results at
    within-tile column 0. g is the tile handle; returns nothing."""
    t = g[:].tensor
    base = g[:].offset + base_off
    ps = g[:].ap[0][0]
    k = K
    while k > 1:
        h = (k + 1) // 2
        s = k - h
        out_ap = bass.AP(t, base, [[ps, P], [K * elem, T], [elem, s], [1, elem]])
        in1_ap = bass.AP(t, base + h * elem,
                         [[ps, P], [K * elem, T], [elem, s], [1, elem]])
        nc.vector.tensor_tensor(out=out_ap, in0=out_ap, in1=in1_ap, op=AL.add)
        k = h


def _build_program(plan, reps=1, mode='full'):
    instrs = plan["instrs"]
    Ks = plan["Ks"]
    n_instr_r = plan["n_instr_r"]
    pool_chunks = plan["pool_chunks"]
    pool_cols = plan["pool_cols"]

    gather_wcols = sum(n_instr_r[r] for r in range(4)) * MAXCOL * 8
    align_wcols = 3 * P * 8  # 3 ranges * 128 padded cols * 8
    pool_wcols = sum(pool_cols) * 8

    nc = bacc.Bacc(None, target_bir_lowering=False, num_devices=C,
                   num_swdge_queues=4)

    # inputs
    deg_in = nc.dram_tensor("deg_tiles", [P, TILES], F32, kind="ExternalInput")
    NBLK = (TILES + 7) // 8
    xt_in = nc.dram_tensor("xt", [32, NBLK * P], F32, kind="ExternalInput")
    gw_in = nc.dram_tensor("gather_w", [P, gather_wcols], I16, kind="ExternalInput")
    aw_in = nc.dram_tensor("align_w", [P, align_wcols], I16, kind="ExternalInput")
    pw_in = nc.dram_tensor("pool_w", [P, pool_wcols], I16, kind="ExternalInput")
    cnt_in = nc.dram_tensor("cnt", [P, 1], F32, kind="ExternalInput")
    ws_in = {}
    ws_in["W1"] = nc.dram_tensor("W1", [32, P], F32, kind="ExternalInput")
    for i in range(2, 6):
        ws_in[f"W{i}"] = nc.dram_tensor(f"W{i}", [P, P], F32, kind="ExternalInput")
    b_in = nc.dram_tensor("bs", [P, 5 * HID], F32, kind="ExternalInput")
    l1w_in = nc.dram_tensor("lin1_w", [HID, HID], F32, kind="ExternalInput")
    l1b_in = nc.dram_tensor("lin1_b", [P, HID], F32, kind="ExternalInput")
    l2w_in = nc.dram_tensor("lin2_w", [HID, 1], F32, kind="ExternalInput")
    l2b_in = nc.dram_tensor("lin2_b", [P, 1], F32, kind="ExternalInput")
    out_t = nc.dram_tensor("out", [P, 1], F32, kind="ExternalOutput")

    # internal DRAM
    table = nc.dram_tensor("table", [TBL_ROWS, ROW], F32)
    rslab = nc.dram_tensor("rslab", [3 * NPAD, ROW], F32)
    ag_in = nc.dram_tensor("ag_in", [NPAD, HID], F32)
    ag_out = nc.dram_tensor("ag_out", [C * NPAD, HID], F32, addr_space="Shared")

    core_id = nc.partition_id_tensor  # noqa: F841  (SPMD id; inputs differ per core)

    with tile.TileContext(nc) as tc:
        import contextlib
        with contextlib.ExitStack() as ctx:
            sbp = ctx.enter_context(tc.tile_pool(name="persist", bufs=1))
            gp = ctx.enter_context(tc.tile_pool(name="g", bufs=6))
            psp = ctx.enter_context(tc.tile_pool(name="ps", bufs=3, space="PSUM"))
            pst = ctx.enter_context(tc.tile_pool(name="pst", bufs=3, space="PSUM"))

            # --- persistent SBUF ---
            idx_g = sbp.tile([P, gather_wcols], I16)
            idx_a = sbp.tile([P, align_wcols], I16)
            idx_p = sbp.tile([P, pool_wcols], I16)
            nc.sync.dma_start(idx_g[:], gw_in[:])
            nc.sync.dma_start(idx_a[:], aw_in[:])
            nc.sync.dma_start(idx_p[:], pw_in[:])

            deg_sb = sbp.tile([P, TILES], F32)
            nc.sync.dma_start(deg_sb[:], deg_in[:])
            dis_sb = sbp.tile([P, TILES], F32)
            nc.scalar.activation(out=dis_sb[:], in_=deg_sb[:],
                                 func=mybir.ActivationFunctionType.Sqrt)
            nc.vector.reciprocal(out=dis_sb[:], in_=dis_sb[:])

            xt_sb = sbp.tile([32, NBLK * P], F32)
            nc.sync.dma_start(xt_sb[:], xt_in[:])

            w_sb = {}
            w_sb[1] = sbp.tile([32, P], F32, tag="w1", name="w1")
            nc.sync.dma_start(w_sb[1][:], ws_in["W1"][:])
            for i in range(2, 6):
                w_sb[i] = sbp.tile([P, P], F32, tag=f"w{i}", name=f"w{i}")
                nc.sync.dma_start(w_sb[i][:], ws_in[f"W{i}"][:])
            b_sb = sbp.tile([P, 5 * HID], F32)
            nc.sync.dma_start(b_sb[:], b_in[:])
            l1w_sb = sbp.tile([HID, HID], F32)
            nc.sync.dma_start(l1w_sb[:], l1w_in[:])
            l1b_sb = sbp.tile([P, HID], F32)
            nc.sync.dma_start(l1b_sb[:], l1b_in[:])
            l2w_sb = sbp.tile([HID, 1], F32)
            nc.sync.dma_start(l2w_sb[:], l2w_in[:])
            l2b_sb = sbp.tile([P, 1], F32)
            nc.sync.dma_start(l2b_sb[:], l2b_in[:])
            cnt_sb = sbp.tile([P, 1], F32)
            nc.sync.dma_start(cnt_sb[:], cnt_in[:])

            ident = sbp.tile([P, P], F32)
            make_identity(nc, ident[:])

            # contiguous per-element dis / b expansions
            dis_exp = sbp.tile([P, TILES * HID], F32)
            de3 = bass.AP(dis_exp[:].tensor, dis_exp[:].offset,
                          [[dis_exp[:].ap[0][0], P], [HID, TILES], [1, HID]])
            db3 = bass.AP(dis_sb[:].tensor, dis_sb[:].offset,
                          [[dis_sb[:].ap[0][0], P], [1, TILES], [0, HID]])
            nc.vector.tensor_copy(out=de3, in_=db3)
            b_exp = sbp.tile([P, TILES * HID], F32)

            y_own = sbp.tile([P, TILES * HID], F32)     # y_l of own dests
            h_sb = sbp.tile([P, TILES * HID], F32)      # h_l of own dests
            slab = [sbp.tile([P, TILES * HID], F32, tag=f"slab{r}", name=f"slab{r}")
                    for r in range(4)]
            pool_slab = sbp.tile([P, HID], F32)
            zeros_sb = sbp.tile([P, 4 * HID], F32)
            nc.vector.memset(zeros_sb[:], 0.0)

            # zero the pad rows of each table window
            for r in range(4):
                dst = bass.AP(table[:].tensor, (WIN * r + PAD_LOCAL) * ROW,
                              [[ROW, P], [ROW * P, 4], [1, HID]])
                src_ap = bass.AP(zeros_sb[:].tensor, zeros_sb[:].offset,
                                 [[zeros_sb[:].ap[0][0], P], [HID, 4], [1, HID]])
                nc.sync.dma_start(dst, src_ap)

            qn = [0]

            def gather(idx_tile, wcol_off, n_idx, out_ap, in_off, in_rows):
                in_ap = bass.AP(table[:].tensor, in_off * ROW,
                                [[ROW, in_rows], [1, HID]])
                _dma_gather(
                    nc.gpsimd,
                    out_ap=out_ap,
                    in_ap=in_ap,
                    idxs_ap=idx_tile[:, wcol_off:wcol_off + n_idx // 16],
                    num_idxs=n_idx,
                    num_idxs_reg=n_idx,
                    elem_size=HID,
                    elem_step=ROW,
                    single_packet=False,
                    queue_num=qn[0] % 4,
                )
                qn[0] += 1

            def mk3(g, T, K):
                a = g[:]
                return bass.AP(a.tensor, a.offset,
                               [[a.ap[0][0], P], [K * HID, T], [1, HID]])

            def epilogue_and_y(layer):
                """h = relu(dis*(S0+S1+S2+S3+y_own)+b); if layer<5 compute
                y' = dis*(h@W_{l+1}) into y_own; write ag_in."""
                s = slab[0][:]
                for r in range(1, 4):
                    nc.vector.tensor_add(out=s, in0=s, in1=slab[r][:])
                nc.vector.tensor_add(out=s, in0=s, in1=y_own[:])
                # * dis (contiguous, pre-expanded)
                nc.vector.tensor_mul(out=s, in0=s, in1=dis_exp[:])
                # + b: broadcast b row over tiles into b_exp once, then add
                boff = (layer - 1) * HID
                be3 = bass.AP(b_exp[:].tensor, b_exp[:].offset,
                              [[b_exp[:].ap[0][0], P], [HID, TILES], [1, HID]])
                bb = bass.AP(b_sb[:].tensor, b_sb[:].offset + boff,
                             [[b_sb[:].ap[0][0], P], [0, TILES], [1, HID]])
                nc.vector.tensor_copy(out=be3, in_=bb)
                nc.vector.tensor_add(out=s, in0=s, in1=b_exp[:])
                # relu -> h
                nc.vector.tensor_scalar(out=h_sb[:], in0=s, scalar1=0.0,
                                        scalar2=None, op0=AL.max)

                if layer < 5:
                    W = w_sb[layer + 1]
                    EB = 3
                    for b0 in range(0, NBLK, EB):
                        nb = min(EB, NBLK - b0)
                        pts, hts, pms, ws_ = [], [], [], []
                        for j in range(nb):
                            b = b0 + j
                            w = min(8, TILES - b * 8) * HID
                            ws_.append(w)
                            pt = pst.tile([P, P], F32, tag="tp", space="PSUM",
                                          name="pt")
                            pts.append(pt)
                            nc.tensor.transpose(
                                out=pt[:w, :],
                                in_=h_sb[:, b * 8 * HID:b * 8 * HID + w],
                                identity=ident[:])
                        for j in range(nb):
                            ht = gp.tile([P, P], F32, tag="ht", name="ht")
                            hts.append(ht)
                            nc.vector.tensor_copy(out=ht[:ws_[j], :],
                                                  in_=pts[j][:ws_[j], :])
                        for j in range(nb):
                            pm = psp.tile([P, P], F32, tag="mmb", space="PSUM",
                                          name="pm")
                            pms.append(pm)
                            nc.tensor.matmul(
                                out=pm[:], lhsT=hts[j][:], rhs=W[:],
                                start=True, stop=True)
                        for j in range(nb):
                            b = b0 + j
                            w = ws_[j]
                            nc.vector.tensor_mul(
                                out=y_own[:, b * 8 * HID:b * 8 * HID + w],
                                in0=pms[j][:, :w],
                                in1=dis_exp[:, b * 8 * HID:b * 8 * HID + w])
                    src_t = y_own
                else:
                    src_t = h_sb
                # write to ag_in [12544, 16]
                a = src_t[:]
                src3 = bass.AP(a.tensor, a.offset,
                               [[a.ap[0][0], P], [HID, TILES], [1, HID]])
                dst3 = bass.AP(ag_in[:].tensor, 0,
                               [[HID, P], [P * HID, TILES], [1, HID]])
                nc.sync.dma_start(dst3, src3)

            def allgather_to_table():
                nc.gpsimd.collective_compute(
                    "AllGather", AL.bypass,
                    replica_groups=[list(range(C))],
                    ins=[ag_in[:]], outs=[ag_out[:]])
                for c in range(C):
                    src_ap = bass.AP(ag_out[:].tensor, c * NPAD * HID,
                                     [[HID, NPAD], [1, HID]])
                    dst_ap = bass.AP(table[:].tensor, _core_base(c) * ROW,
                                     [[ROW, NPAD], [1, HID]])
                    nc.sync.dma_start(dst_ap, src_ap)

            def layer1_y():
                EB = 3
                for b0 in range(0, NBLK, EB):
                    nb = min(EB, NBLK - b0)
                    pms, ws_ = [], []
                    for j in range(nb):
                        b = b0 + j
                        ws_.append(min(8, TILES - b * 8) * HID)
                        pm = psp.tile([P, P], F32, tag="mmb", space="PSUM",
                                      name="pm")
                        pms.append(pm)
                        nc.tensor.matmul(
                            out=pm[:], lhsT=xt_sb[:, b * P:(b + 1) * P],
                            rhs=w_sb[1][:], start=True, stop=True)
                    for j in range(nb):
                        b = b0 + j
                        w = ws_[j]
                        nc.vector.tensor_mul(
                            out=y_own[:, b * 8 * HID:b * 8 * HID + w],
                            in0=pms[j][:, :w],
                            in1=dis_exp[:, b * 8 * HID:b * 8 * HID + w])
                a = y_own[:]
                src3 = bass.AP(a.tensor, a.offset,
                               [[a.ap[0][0], P], [HID, TILES], [1, HID]])
                dst3 = bass.AP(ag_in[:].tensor, 0,
                               [[HID, P], [P * HID, TILES], [1, HID]])
                nc.sync.dma_start(dst3, src3)

            # wrapped-col offset of each range's token block
            g_woff = [0]
            for r in range(4):
                g_woff.append(g_woff[-1] + n_instr_r[r] * MAXCOL * 8)

            def message_pass_gr():
                for r, ii, t0, T, K in instrs:
                    g = gp.tile([P, MAXCOL * HID], F32, tag="g", name="g")
                    out3 = bass.AP(g[:].tensor, g[:].offset,
                                   [[g[:].ap[0][0], P], [HID, MAXCOL], [1, HID]])
                    gather(idx_g, g_woff[r] + ii * MAXCOL * 8, MAXCOL * P, out3,
                           WIN * r, WIN)
                    k = K
                    while k > 1:
                        h = (k + 1) // 2
                        srcn = k - h
                        nc.vector.tensor_add(
                            out=g[:, :srcn * T * HID],
                            in0=g[:, :srcn * T * HID],
                            in1=g[:, h * T * HID:(h + srcn) * T * HID])
                        k = h

            def message_pass_gonly():
                for r, ii, secs in instrs:
                    g = gp.tile([P, MAXCOL * HID], F32, tag="g", name="g")
                    out3 = bass.AP(g[:].tensor, g[:].offset,
                                   [[g[:].ap[0][0], P], [HID, MAXCOL], [1, HID]])
                    gather(idx_g, g_woff[r] + ii * MAXCOL * 8, MAXCOL * P, out3,
                           WIN * r, WIN)

            def message_pass():
                """gathers + reduces into slab[0..3]; slabs 1-3 via rslab."""
                BATCH = 5
                for bi in range(0, len(instrs), BATCH):
                    batch = instrs[bi:bi + BATCH]
                    gs = []
                    for r, ii, t0, T, K in batch:
                        g = gp.tile([P, MAXCOL * HID], F32, tag="g", name="g")
                        gs.append(g)
                        out3 = bass.AP(g[:].tensor, g[:].offset,
                                       [[g[:].ap[0][0], P], [HID, MAXCOL], [1, HID]])
                        gather(idx_g, g_woff[r] + ii * MAXCOL * 8, MAXCOL * P,
                               out3, WIN * r, WIN)
                    for g, (r, ii, t0, T, K) in zip(gs, batch):
                        k = K
                        while k > 1:
                            h = (k + 1) // 2
                            srcn = k - h
                            nc.vector.tensor_add(
                                out=g[:, :srcn * T * HID],
                                in0=g[:, :srcn * T * HID],
                                in1=g[:, h * T * HID:(h + srcn) * T * HID])
                            k = h
                        nc.vector.tensor_copy(
                            out=slab[r][:, t0 * HID:(t0 + T) * HID],
                            in_=g[:, :T * HID])
                # slabs 1-3 -> rslab -> align gather back into slab[r]
                for r in range(1, 4):
                    sl = slab[r][:]
                    src3 = bass.AP(sl.tensor, sl.offset,
                                   [[sl.ap[0][0], P], [HID, TILES], [1, HID]])
                    dst3 = bass.AP(rslab[:].tensor, (r - 1) * NPAD * ROW,
                                   [[ROW, P], [P * ROW, TILES], [1, HID]])
                    nc.sync.dma_start(dst3, src3)
                for r in range(1, 4):
                    awoff = (r - 1) * (P * 8)
                    done = 0
                    while done < TILES:
                        ntl = min(MAXCOL, TILES - done)
                        n_idx = ntl * P
                        ga = gp.tile([P, MAXCOL * HID], F32, tag="g", name="g")
                        out3 = bass.AP(ga[:].tensor, ga[:].offset,
                                       [[ga[:].ap[0][0], P], [HID, ntl], [1, HID]])
                        in_ap = bass.AP(rslab[:].tensor, (r - 1) * NPAD * ROW,
                                        [[ROW, NPAD], [1, HID]])
                        _dma_gather(
                            nc.gpsimd, out_ap=out3, in_ap=in_ap,
                            idxs_ap=idx_a[:, awoff + done * 8:
                                          awoff + (done + ntl) * 8],
                            num_idxs=n_idx, num_idxs_reg=n_idx,
                            elem_size=HID, elem_step=ROW,
                            single_packet=False, queue_num=qn[0] % 4)
                        qn[0] += 1
                        sl = slab[r][:]
                        dst_ap = bass.AP(sl.tensor, sl.offset + done * HID,
                                         [[sl.ap[0][0], P], [HID, ntl], [1, HID]])
                        nc.vector.tensor_copy(out=dst_ap, in_=out3)
                        done += ntl

            def pooling_and_head():
                first = True
                woff = 0
                for r, coff, K, acc in pool_chunks:
                    n_idx = K * P
                    g = gp.tile([P, MAXCOL * HID], F32, tag="g")
                    out3 = bass.AP(g[:].tensor, g[:].offset,
                                   [[g[:].ap[0][0], P], [HID, K], [1, HID]])
                    gather(idx_p, woff, n_idx, out3, WIN * r, WIN)
                    woff += MAXCOL * 8
                    k = K
                    while k > 1:
                        h = (k + 1) // 2
                        srcn = k - h
                        nc.vector.tensor_add(
                            out=g[:, :srcn * HID], in0=g[:, :srcn * HID],
                            in1=g[:, h * HID:(h + srcn) * HID])
                        k = h
                    if first:
                        nc.vector.tensor_copy(out=pool_slab[:], in_=g[:, :HID])
                        first = False
                    else:
                        nc.vector.tensor_add(out=pool_slab[:], in0=pool_slab[:],
                                             in1=g[:, :HID])
                # mean
                rcp = gp.tile([P, 1], F32, tag="rcp")
                nc.vector.reciprocal(out=rcp[:], in_=cnt_sb[:])
                nc.vector.tensor_scalar(out=pool_slab[:], in0=pool_slab[:],
                                        scalar1=rcp[:], scalar2=None,
                                        op0=AL.mult)

                def rrelu(ap):
                    pos = gp.tile([P, HID], F32, tag="rr1")
                    nc.vector.tensor_scalar(out=pos[:, :ap.shape[1]], in0=ap,
                                            scalar1=0.0, scalar2=None, op0=AL.max)
                    nc.vector.tensor_scalar(out=ap, in0=ap, scalar1=0.0,
                                            scalar2=RRELU_SLOPE, op0=AL.min,
                                            op1=AL.mult)
                    nc.vector.tensor_add(out=ap, in0=ap,
                                         in1=pos[:, :ap.shape[1]])

                # lin1
                pt = pst.tile([P, P], F32, tag="tp", space="PSUM")
                nc.tensor.transpose(out=pt[:HID, :], in_=pool_slab[:],
                                    identity=ident[:])
                gt = gp.tile([HID, P], F32, tag="gt")
                nc.vector.tensor_copy(out=gt[:], in_=pt[:HID, :])
                pm = pst.tile([P, HID], F32, tag="tp", space="PSUM", name="pmp")
                nc.tensor.matmul(out=pm[:], lhsT=gt[:], rhs=l1w_sb[:],
                                 start=True, stop=True)
                g1 = gp.tile([P, HID], F32, tag="g1")
                nc.vector.tensor_add(out=g1[:], in0=pm[:], in1=l1b_sb[:])
                rrelu(g1[:])
                # lin2
                pt2 = pst.tile([P, P], F32, tag="tp", space="PSUM")
                nc.tensor.transpose(out=pt2[:HID, :], in_=g1[:],
                                    identity=ident[:])
                gt2 = gp.tile([HID, P], F32, tag="gt")
                nc.vector.tensor_copy(out=gt2[:], in_=pt2[:HID, :])
                pm2 = pst.tile([P, 1], F32, tag="tp", space="PSUM", name="pmp2")
                nc.tensor.matmul(out=pm2[:], lhsT=gt2[:], rhs=l2w_sb[:],
                                 start=True, stop=True)
                g2 = gp.tile([P, 1], F32, tag="g2")
                nc.vector.tensor_add(out=g2[:], in0=pm2[:], in1=l2b_sb[:])
                rrelu(g2[:])
                nc.sync.dma_start(out_t[:], g2[:])

            if mode == "full":
                for _ in range(reps):
                    layer1_y()
                    allgather_to_table()
                    for layer in range(1, 6):
                        message_pass()
                        epilogue_and_y(layer)
                        if layer < 5:
                            allgather_to_table()
                    # h5 -> table
                    allgather_to_table()
                    pooling_and_head()
            elif mode == "gr":
                layer1_y()
                allgather_to_table()
                for _ in range(reps):
                    for _l in range(5):
                        message_pass_gr()
                pooling_and_head()
            elif mode == "gonly":
                layer1_y()
                allgather_to_table()
                for _ in range(reps):
                    for _l in range(5):
                        message_pass_gonly()
                pooling_and_head()
            elif mode == "gathers":
                layer1_y()
                allgather_to_table()
                for _ in range(reps):
                    for _l in range(5):
                        message_pass()
                pooling_and_head()
            elif mode == "ag":
                layer1_y()
                for _ in range(reps):
                    for _l in range(6):
                        allgather_to_table()
                pooling_and_head()
            elif mode == "epilogue":
                layer1_y()
                allgather_to_table()
                message_pass()
                for _ in range(reps):
                    for layer in range(1, 6):
                        epilogue_and_y(layer)
                pooling_and_head()

    nc.finalize()
    return nc


def kernel(x, edge_index, batch, W1, b1, W2, b2, W3, b3, W4, b4, W5, b5,
           lin1_w, lin1_b, lin2_w, lin2_b, _reps=1, _prebuilt=None):
    per_core, plan = _preprocess(x, edge_index, batch)
    nc = _prebuilt if _prebuilt is not None else _build_program(plan, reps=_reps)

    bs = np.concatenate([np.asarray(b, np.float32) for b in
                         (b1, b2, b3, b4, b5)]).reshape(1, 5 * HID)
    bs = np.repeat(bs, P, axis=0).copy()
    l1b = np.repeat(np.asarray(lin1_b, np.float32).reshape(1, HID), P, 0).copy()
    l2b = np.repeat(np.asarray(lin2_b, np.float32).reshape(1, 1), P, 0).copy()

    in_maps = []
    for c in range(C):
        pc = per_core[c]
        in_maps.append({
            "deg_tiles": pc["deg_tiles"].astype(np.float32),
            "xt": pc["xt"],
            "gather_w": pc["gather_w"],
            "align_w": pc["align_w"],
            "pool_w": pc["pool_w"],
            "cnt": pc["cnt"],
            "W1": np.kron(np.eye(8, dtype=np.float32), np.asarray(W1, np.float32)),
            "W2": np.kron(np.eye(8, dtype=np.float32), np.asarray(W2, np.float32)),
            "W3": np.kron(np.eye(8, dtype=np.float32), np.asarray(W3, np.float32)),
            "W4": np.kron(np.eye(8, dtype=np.float32), np.asarray(W4, np.float32)),
            "W5": np.kron(np.eye(8, dtype=np.float32), np.asarray(W5, np.float32)),
            "bs": bs,
            "lin1_w": np.asarray(lin1_w, np.float32),
            "lin1_b": l1b,
            "lin2_w": np.asarray(lin2_w, np.float32),
            "lin2_b": l2b,
        })

    res = run_bass_via_pjrt(nc, in_maps, n_cores=C)
    out = np.zeros((N_GRAPHS, 1), dtype=np.float32)
    for c in range(C):
        out[c * GPC:(c + 1) * GPC, 0] = res[c]["out"][:GPC, 0]
    return out



# revision 5
# speedup vs baseline: 98.3254x; 98.3254x over previous
"""GCN (5-layer) + global mean pool + MLP head on 8 trn2 NeuronCores.

Strategy (dest-sharded, AllGather):
  - Factorize GCN norm: with dis = rsqrt(deg), y = dis * (h @ W), the layer is
      h'[v] = relu(dis[v] * (sum_{u->v} y[u] + y[v]) + b)
    so message passing is a pure gather+sum of 64B rows.
  - Nodes are sharded across 8 cores (12500 each). Each core gathers its
    in-edges' source rows from a DRAM y-table via dma_gather (int16 indices,
    64B elements from a 256B-strided table), reduces slot-groups on DVE,
    applies the epilogue, computes its y' slice, and AllGathers y' each layer.
  - The y-table is split in 4 windows of 25600 rows so indices fit int16;
    per (core, source-window) destination orderings are degree-sorted to
    minimize slot padding, then realigned with 3 small index gathers.
"""
import inspect
import re

import numpy as np

import concourse.bass as bass
import concourse.bacc as bacc
import concourse.tile as tile
import concourse.mybir as mybir
from concourse.bass2jax import run_bass_via_pjrt
from concourse.masks import make_identity

F32 = mybir.dt.float32
I16 = mybir.dt.int16
AL = mybir.AluOpType

N_NODES = 100000
N_EDGES = 3200000
N_GRAPHS = 1000
HID = 16
C = 8                    # cores
NPC = N_NODES // C       # 12500 nodes per core
P = 128
TILES = 98               # ceil(12500/128)
NPAD = TILES * P         # 12544
WIN = 25600              # table window rows (int16-addressable)
PAD_LOCAL = 2 * NPAD     # zero rows at window-local [25088, 25600)
TBL_ROWS = 4 * WIN       # 102400
ROW = 64                 # table row stride in f32 (256B)
GPC = N_GRAPHS // C      # 125 graphs per core
MAXCOL = 32              # max token columns per gather instruction (4096 idx)
RRELU_SLOPE = (1.0 / 8.0 + 1.0 / 3.0) / 2.0

# table row base for core c: window c//2, upper/lower half
def _core_base(c):
    return WIN * (c // 2) + NPAD * (c % 2)


def _make_patched_dma_gather():
    """dma_gather with the elem_size%256 assert dropped (the 256B constraint
    is on the row stride; 64B payloads from a 256B-strided table work)."""
    src = inspect.getsource(bass.BassGpSimd.dma_gather)
    src = src.replace(
        "assert (\n            elem_size_bytes > 0 and elem_size_bytes % 256 == 0\n        )  # transpose restriction",
        "assert elem_size_bytes > 0")
    src = re.sub(r"^    def dma_gather", "def dma_gather_patched", src, flags=re.M)
    src = "\n".join(l[4:] if l.startswith("    ") else l for l in src.splitlines())
    ns = dict(bass.__dict__)
    exec(src, ns)
    return ns["dma_gather_patched"]


_dma_gather = _make_patched_dma_gather()


def _wrap_idx(tokens):
    """[128, cols] token array (token i at (i%128, i//128)) -> [128, cols*8]
    int16 wrapped index layout (idx i at (i%16, i//16), replicated x8)."""
    p, cols = tokens.shape
    assert p == P
    flat = tokens.T.reshape(-1)                  # i = col*128 + p
    w16 = flat.reshape(-1, 16).T                 # [16, cols*8]
    return np.tile(w16, (8, 1)).astype(np.int16)


def _preprocess(x, edge_index, batch):
    """Build per-core device inputs + shared (cross-core identical) plan."""
    src = np.asarray(edge_index[0], dtype=np.int64)
    dst = np.asarray(edge_index[1], dtype=np.int64)
    batch = np.asarray(batch, dtype=np.int64)
    x = np.asarray(x, dtype=np.float32)

    deg = np.bincount(dst, minlength=N_NODES).astype(np.float32) + 1.0

    dcore = dst // NPC
    srng = src // WIN  # 0 only for now; real range needs table rows (below)

    # per (core, range) in-degree of each dest; range of an edge = source's
    # table window = (source core)//2
    src_core = src // NPC
    rng = src_core // 2
    key = dst * 4 + rng
    cnt4 = np.bincount(key, minlength=N_NODES * 4).reshape(N_NODES, 4)

    # destination orderings: perm[c][r] = local dest ids in processing order
    perm = [[None] * 4 for _ in range(C)]
    rank_in_perm = [[None] * 4 for _ in range(C)]
    for c in range(C):
        lo = c * NPC
        for r in range(4):
            d = cnt4[lo:lo + NPC, r]
            order = np.argsort(-d, kind="stable")
            perm[c][r] = order
            rk = np.empty(NPC, dtype=np.int64)
            rk[order] = np.arange(NPC)
            rank_in_perm[c][r] = rk

    # canonical (sigma) order per core = perm[c][0]; table row of each node
    row_of_node = np.empty(N_NODES, dtype=np.int64)
    for c in range(C):
        lo = c * NPC
        row_of_node[lo + perm[c][0]] = _core_base(c) + np.arange(NPC)

    # K structure per (range, tile): cross-core max of tile-max degree
    Ks = np.zeros((4, TILES), dtype=np.int64)
    for c in range(C):
        for r in range(4):
            d_sorted = cnt4[c * NPC:(c + 1) * NPC, r][perm[c][r]]
            d_pad = np.concatenate([d_sorted, np.zeros(NPAD - NPC, np.int64)])
            tile_max = d_pad.reshape(TILES, P).max(axis=1)
            Ks[r] = np.maximum(Ks[r], tile_max)
    Ks = np.maximum(Ks, 1)
    assert Ks.max() <= MAXCOL, f"tile K {Ks.max()} exceeds {MAXCOL}"

    # instructions per range: each = MAXCOL token columns (4096 idx), ONE K,
    # T = n tiles, slot-major columns (col = slot*T + tile_j) so reduces and
    # copies are contiguous. entry: (r, instr_idx_in_range, t_start, T, K)
    instrs = []
    tile_base = [np.zeros(TILES, np.int64) for _ in range(4)]  # instr base col
    tile_T = [np.zeros(TILES, np.int64) for _ in range(4)]
    tile_j = [np.zeros(TILES, np.int64) for _ in range(4)]
    n_instr_r = [0] * 4
    for r in range(4):
        ii = 0
        t = 0
        while t < TILES:
            K = int(Ks[r][t])
            cap = MAXCOL // K
            T = 1
            while (T < cap and t + T < TILES and int(Ks[r][t + T]) == K):
                T += 1
            for j in range(T):
                tile_base[r][t + j] = ii * MAXCOL
                tile_T[r][t + j] = T
                tile_j[r][t + j] = j
            instrs.append((r, ii, t, T, K))
            ii += 1
            t += T
        n_instr_r[r] = ii

    # token arrays per core per range: [128, COLS_r] window-local row indices
    tok = [[np.full((P, n_instr_r[r] * MAXCOL), PAD_LOCAL, dtype=np.int64)
            for r in range(4)] for _ in range(C)]
    erank = np.empty(N_EDGES, dtype=np.int64)  # dest rank in perm[dcore][rng]
    for c in range(C):
        m = dcore == c
        for r in range(4):
            mm = m & (rng == r)
            erank[mm] = rank_in_perm[c][r][dst[mm] - c * NPC]
    # slot index of each edge within its (dest, range) group
    order = np.lexsort((erank, rng, dcore))
    so_dcore, so_rng, so_rank = dcore[order], rng[order], erank[order]
    gkey = (so_dcore * 4 + so_rng) * NPC + so_rank
    starts = np.concatenate([[True], gkey[1:] != gkey[:-1]])
    gidx = np.cumsum(starts) - 1
    first = np.flatnonzero(starts)
    slot = np.arange(len(order)) - first[gidx]
    so_locrow = (row_of_node[src[order]]) % WIN
    for c in range(C):
        m = so_dcore == c
        for r in range(4):
            mm = m & (so_rng == r)
            rk = so_rank[mm]
            tl = rk // P
            col = tile_base[r][tl] + slot[mm] * tile_T[r][tl] + tile_j[r][tl]
            tok[c][r][rk % P, col] = so_locrow[mm]

    # align-gather indices (ranges 1..3): position i (canonical) -> rank in
    # perm_r of the dest at canonical position i
    align_idx = [[None] * 4 for _ in range(C)]
    for c in range(C):
        for r in range(1, 4):
            ai = np.arange(NPAD, dtype=np.int64)
            ai[:NPC] = rank_in_perm[c][r][perm[c][0]]
            align_idx[c][r] = ai

    # pooling: graph g -> core g // GPC ; member rows per (core, graph, range)
    g_of_node = batch
    node_rows = row_of_node
    node_rng = node_rows // WIN
    pkey = (g_of_node * 4 + node_rng)
    pcnt = np.bincount(pkey, minlength=N_GRAPHS * 4).reshape(N_GRAPHS, 4)
    Kp = np.zeros(4, dtype=np.int64)
    for r in range(4):
        Kp[r] = max(1, pcnt[:, r].max())
    # chunk pool ranges into <=MAXCOL col chunks (accumulate extra chunks)
    pool_chunks = []  # (r, col_off_in_pool_r, K_chunk, accum)
    pool_cols = []
    for r in range(4):
        off = 0
        K = int(Kp[r])
        acc = False
        while K > 0:
            k = min(K, MAXCOL)
            pool_chunks.append((r, off, k, acc))
            off += MAXCOL
            K -= k
            acc = True
        pool_cols.append(off)

    ptok = [[np.full((P, pool_cols[r]), PAD_LOCAL, dtype=np.int64)
             for r in range(4)] for _ in range(C)]
    porder = np.lexsort((g_of_node, node_rng))
    po_g, po_rng = g_of_node[porder], node_rng[porder]
    pk = po_g * 4 + po_rng
    pstarts = np.concatenate([[True], pk[1:] != pk[:-1]])
    pgidx = np.cumsum(pstarts) - 1
    pfirst = np.flatnonzero(pstarts)
    pslot = np.arange(len(porder)) - pfirst[pgidx]
    po_locrow = node_rows[porder] % WIN
    for r in range(4):
        m = po_rng == r
        g = po_g[m]
        c = g // GPC
        part = g % GPC
        sl = pslot[m]
        col = (sl // MAXCOL) * MAXCOL + (sl % MAXCOL)  # == sl; chunks stride MAXCOL
        for cc in range(C):
            mm = c == cc
            ptok[cc][r][part[mm], col[mm]] = po_locrow[m][mm]

    cnt_graph = np.bincount(batch, minlength=N_GRAPHS).astype(np.float32)
    cnt_graph = np.maximum(cnt_graph, 1.0)

    # per-core host arrays
    per_core = []
    for c in range(C):
        lo = c * NPC
        sigma = perm[c][0]
        nodes_sigma = lo + sigma  # node id at canonical position i
        deg_pad = np.ones(NPAD, dtype=np.float32)
        deg_pad[:NPC] = deg[nodes_sigma]
        deg_tiles = deg_pad.reshape(TILES, P).T.copy()  # [128, 98]

        nblk = (TILES + 7) // 8
        xp = np.zeros((nblk * 8 * P, 4), dtype=np.float32)
        xp[:NPC] = x[nodes_sigma]
        xt = xp.reshape(nblk, 8, P, 4).transpose(1, 3, 0, 2).reshape(32, nblk * P).copy()

        gather_w = np.concatenate(
            [_wrap_idx(tok[c][r]) for r in range(4)], axis=1)
        align_w_parts = []
        for r in range(1, 4):
            a = np.zeros((P, P), dtype=np.int64)
            a[:, :TILES] = align_idx[c][r].reshape(TILES, P).T
            align_w_parts.append(_wrap_idx(a))
        align_w = np.concatenate(align_w_parts, axis=1)
        pool_w = np.concatenate(
            [_wrap_idx(ptok[c][r]) for r in range(4)], axis=1)

        cnt_c = np.ones((P, 1), dtype=np.float32)
        cnt_c[:GPC, 0] = cnt_graph[c * GPC:(c + 1) * GPC]

        per_core.append(dict(
            deg_tiles=deg_tiles, xt=xt, gather_w=gather_w,
            align_w=align_w, pool_w=pool_w, cnt=cnt_c))

    plan = dict(instrs=instrs, Ks=Ks, n_instr_r=n_instr_r,
                pool_chunks=pool_chunks, pool_cols=pool_cols)
    return per_core, plan


def _reduce_equalK(nc, g, T, K, elem=HID, base_off=0):
    """In-place slot reduce of G viewed as [128, T, K, elem] -> results at
    within-tile column 0. g is the tile handle; returns nothing."""
    t = g[:].tensor
    base = g[:].offset + base_off
    ps = g[:].ap[0][0]
    k = K
    while k > 1:
        h = (k + 1) // 2
        s = k - h
        out_ap = bass.AP(t, base, [[ps, P], [K * elem, T], [elem, s], [1, elem]])
        in1_ap = bass.AP(t, base + h * elem,
                         [[ps, P], [K * elem, T], [elem, s], [1, elem]])
        nc.vector.tensor_tensor(out=out_ap, in0=out_ap, in1=in1_ap, op=AL.add)
        k = h


def _build_program(plan, reps=1, mode='full'):
    instrs = plan["instrs"]
    Ks = plan["Ks"]
    n_instr_r = plan["n_instr_r"]
    pool_chunks = plan["pool_chunks"]
    pool_cols = plan["pool_cols"]

    gather_wcols = sum(n_instr_r[r] for r in range(4)) * MAXCOL * 8
    align_wcols = 3 * P * 8  # 3 ranges * 128 padded cols * 8
    pool_wcols = sum(pool_cols) * 8

    nc = bacc.Bacc(None, target_bir_lowering=False, num_devices=C,
                   num_swdge_queues=4)

    # inputs
    deg_in = nc.dram_tensor("deg_tiles", [P, TILES], F32, kind="ExternalInput")
    NBLK = (TILES + 7) // 8
    xt_in = nc.dram_tensor("xt", [32, NBLK * P], F32, kind="ExternalInput")
    gw_in = nc.dram_tensor("gather_w", [P, gather_wcols], I16, kind="ExternalInput")
    aw_in = nc.dram_tensor("align_w", [P, align_wcols], I16, kind="ExternalInput")
    pw_in = nc.dram_tensor("pool_w", [P, pool_wcols], I16, kind="ExternalInput")
    cnt_in = nc.dram_tensor("cnt", [P, 1], F32, kind="ExternalInput")
    ws_in = {}
    ws_in["W1"] = nc.dram_tensor("W1", [32, P], F32, kind="ExternalInput")
    for i in range(2, 6):
        ws_in[f"W{i}"] = nc.dram_tensor(f"W{i}", [P, P], F32, kind="ExternalInput")
    b_in = nc.dram_tensor("bs", [P, 5 * HID], F32, kind="ExternalInput")
    l1w_in = nc.dram_tensor("lin1_w", [HID, HID], F32, kind="ExternalInput")
    l1b_in = nc.dram_tensor("lin1_b", [P, HID], F32, kind="ExternalInput")
    l2w_in = nc.dram_tensor("lin2_w", [HID, 1], F32, kind="ExternalInput")
    l2b_in = nc.dram_tensor("lin2_b", [P, 1], F32, kind="ExternalInput")
    out_t = nc.dram_tensor("out", [P, 1], F32, kind="ExternalOutput")

    # internal DRAM
    table = nc.dram_tensor("table", [TBL_ROWS, ROW], F32)
    rslab = nc.dram_tensor("rslab", [3 * NPAD, ROW], F32)
    ag_in = nc.dram_tensor("ag_in", [NPAD, HID], F32)
    ag_out = nc.dram_tensor("ag_out", [C * NPAD, HID], F32, addr_space="Shared")

    core_id = nc.partition_id_tensor  # noqa: F841  (SPMD id; inputs differ per core)

    with tile.TileContext(nc) as tc:
        import contextlib
        with contextlib.ExitStack() as ctx:
            sbp = ctx.enter_context(tc.tile_pool(name="persist", bufs=1))
            gp = ctx.enter_context(tc.tile_pool(name="g", bufs=6))
            psp = ctx.enter_context(tc.tile_pool(name="ps", bufs=3, space="PSUM"))
            pst = ctx.enter_context(tc.tile_pool(name="pst", bufs=3, space="PSUM"))

            # --- persistent SBUF ---
            idx_g = sbp.tile([P, gather_wcols], I16)
            idx_a = sbp.tile([P, align_wcols], I16)
            idx_p = sbp.tile([P, pool_wcols], I16)
            nc.sync.dma_start(idx_g[:], gw_in[:])
            nc.sync.dma_start(idx_a[:], aw_in[:])
            nc.sync.dma_start(idx_p[:], pw_in[:])

            deg_sb = sbp.tile([P, TILES], F32)
            nc.sync.dma_start(deg_sb[:], deg_in[:])
            dis_sb = sbp.tile([P, TILES], F32)
            nc.scalar.activation(out=dis_sb[:], in_=deg_sb[:],
                                 func=mybir.ActivationFunctionType.Sqrt)
            nc.vector.reciprocal(out=dis_sb[:], in_=dis_sb[:])

            xt_sb = sbp.tile([32, NBLK * P], F32)
            nc.sync.dma_start(xt_sb[:], xt_in[:])

            w_sb = {}
            w_sb[1] = sbp.tile([32, P], F32, tag="w1", name="w1")
            nc.sync.dma_start(w_sb[1][:], ws_in["W1"][:])
            for i in range(2, 6):
                w_sb[i] = sbp.tile([P, P], F32, tag=f"w{i}", name=f"w{i}")
                nc.sync.dma_start(w_sb[i][:], ws_in[f"W{i}"][:])
            b_sb = sbp.tile([P, 5 * HID], F32)
            nc.sync.dma_start(b_sb[:], b_in[:])
            l1w_sb = sbp.tile([HID, HID], F32)
            nc.sync.dma_start(l1w_sb[:], l1w_in[:])
            l1b_sb = sbp.tile([P, HID], F32)
            nc.sync.dma_start(l1b_sb[:], l1b_in[:])
            l2w_sb = sbp.tile([HID, 1], F32)
            nc.sync.dma_start(l2w_sb[:], l2w_in[:])
            l2b_sb = sbp.tile([P, 1], F32)
            nc.sync.dma_start(l2b_sb[:], l2b_in[:])
            cnt_sb = sbp.tile([P, 1], F32)
            nc.sync.dma_start(cnt_sb[:], cnt_in[:])

            ident = sbp.tile([P, P], F32)
            make_identity(nc, ident[:])

            # contiguous per-element dis / b expansions
            dis_exp = sbp.tile([P, TILES * HID], F32)
            de3 = bass.AP(dis_exp[:].tensor, dis_exp[:].offset,
                          [[dis_exp[:].ap[0][0], P], [HID, TILES], [1, HID]])
            db3 = bass.AP(dis_sb[:].tensor, dis_sb[:].offset,
                          [[dis_sb[:].ap[0][0], P], [1, TILES], [0, HID]])
            nc.vector.tensor_copy(out=de3, in_=db3)
            b_exp = sbp.tile([P, TILES * HID], F32)

            y_own = sbp.tile([P, TILES * HID], F32)     # y_l of own dests
            h_sb = sbp.tile([P, TILES * HID], F32)      # h_l of own dests
            slab = [sbp.tile([P, TILES * HID], F32, tag=f"slab{r}", name=f"slab{r}")
                    for r in range(4)]
            pool_slab = sbp.tile([P, HID], F32)
            zeros_sb = sbp.tile([P, 4 * HID], F32)
            nc.vector.memset(zeros_sb[:], 0.0)

            # zero the pad rows of each table window
            for r in range(4):
                dst = bass.AP(table[:].tensor, (WIN * r + PAD_LOCAL) * ROW,
                              [[ROW, P], [ROW * P, 4], [1, HID]])
                src_ap = bass.AP(zeros_sb[:].tensor, zeros_sb[:].offset,
                                 [[zeros_sb[:].ap[0][0], P], [HID, 4], [1, HID]])
                nc.sync.dma_start(dst, src_ap)

            qn = [0]

            def gather(idx_tile, wcol_off, n_idx, out_ap, in_off, in_rows):
                in_ap = bass.AP(table[:].tensor, in_off * ROW,
                                [[ROW, in_rows], [1, HID]])
                _dma_gather(
                    nc.gpsimd,
                    out_ap=out_ap,
                    in_ap=in_ap,
                    idxs_ap=idx_tile[:, wcol_off:wcol_off + n_idx // 16],
                    num_idxs=n_idx,
                    num_idxs_reg=n_idx,
                    elem_size=HID,
                    elem_step=ROW,
                    single_packet=False,
                    queue_num=qn[0] % 4,
                )
                qn[0] += 1

            def mk3(g, T, K):
                a = g[:]
                return bass.AP(a.tensor, a.offset,
                               [[a.ap[0][0], P], [K * HID, T], [1, HID]])

            def epilogue_and_y(layer):
                """h = relu(dis*(S0+S1+S2+S3+y_own)+b); if layer<5 compute
                y' = dis*(h@W_{l+1}) into y_own; write ag_in."""
                s = slab[0][:]
                for r in range(1, 4):
                    nc.vector.tensor_add(out=s, in0=s, in1=slab[r][:])
                nc.vector.tensor_add(out=s, in0=s, in1=y_own[:])
                # * dis (contiguous, pre-expanded)
                nc.vector.tensor_mul(out=s, in0=s, in1=dis_exp[:])
                # + b: broadcast b row over tiles into b_exp once, then add
                boff = (layer - 1) * HID
                be3 = bass.AP(b_exp[:].tensor, b_exp[:].offset,
                              [[b_exp[:].ap[0][0], P], [HID, TILES], [1, HID]])
                bb = bass.AP(b_sb[:].tensor, b_sb[:].offset + boff,
                             [[b_sb[:].ap[0][0], P], [0, TILES], [1, HID]])
                nc.vector.tensor_copy(out=be3, in_=bb)
                nc.vector.tensor_add(out=s, in0=s, in1=b_exp[:])
                # relu -> h
                nc.vector.tensor_scalar(out=h_sb[:], in0=s, scalar1=0.0,
                                        scalar2=None, op0=AL.max)

                if layer < 5:
                    W = w_sb[layer + 1]
                    EB = 3
                    for b0 in range(0, NBLK, EB):
                        nb = min(EB, NBLK - b0)
                        pts, hts, pms, ws_ = [], [], [], []
                        for j in range(nb):
                            b = b0 + j
                            w = min(8, TILES - b * 8) * HID
                            ws_.append(w)
                            pt = pst.tile([P, P], F32, tag="tp", space="PSUM",
                                          name="pt")
                            pts.append(pt)
                            nc.tensor.transpose(
                                out=pt[:w, :],
                                in_=h_sb[:, b * 8 * HID:b * 8 * HID + w],
                                identity=ident[:])
                        for j in range(nb):
                            ht = gp.tile([P, P], F32, tag="ht", name="ht")
                            hts.append(ht)
                            nc.vector.tensor_copy(out=ht[:ws_[j], :],
                                                  in_=pts[j][:ws_[j], :])
                        for j in range(nb):
                            pm = psp.tile([P, P], F32, tag="mmb", space="PSUM",
                                          name="pm")
                            pms.append(pm)
                            nc.tensor.matmul(
                                out=pm[:], lhsT=hts[j][:], rhs=W[:],
                                start=True, stop=True)
                        for j in range(nb):
                            b = b0 + j
                            w = ws_[j]
                            nc.vector.tensor_mul(
                                out=y_own[:, b * 8 * HID:b * 8 * HID + w],
                                in0=pms[j][:, :w],
                                in1=dis_exp[:, b * 8 * HID:b * 8 * HID + w])
                    src_t = y_own
                else:
                    src_t = h_sb
                # write to ag_in [12544, 16]
                a = src_t[:]
                src3 = bass.AP(a.tensor, a.offset,
                               [[a.ap[0][0], P], [HID, TILES], [1, HID]])
                dst3 = bass.AP(ag_in[:].tensor, 0,
                               [[HID, P], [P * HID, TILES], [1, HID]])
                nc.sync.dma_start(dst3, src3)

            def allgather_to_table():
                nc.gpsimd.collective_compute(
                    "AllGather", AL.bypass,
                    replica_groups=[list(range(C))],
                    ins=[ag_in[:]], outs=[ag_out[:]])
                for c in range(C):
                    src_ap = bass.AP(ag_out[:].tensor, c * NPAD * HID,
                                     [[HID, NPAD], [1, HID]])
                    dst_ap = bass.AP(table[:].tensor, _core_base(c) * ROW,
                                     [[ROW, NPAD], [1, HID]])
                    nc.sync.dma_start(dst_ap, src_ap)

            def layer1_y():
                EB = 3
                for b0 in range(0, NBLK, EB):
                    nb = min(EB, NBLK - b0)
                    pms, ws_ = [], []
                    for j in range(nb):
                        b = b0 + j
                        ws_.append(min(8, TILES - b * 8) * HID)
                        pm = psp.tile([P, P], F32, tag="mmb", space="PSUM",
                                      name="pm")
                        pms.append(pm)
                        nc.tensor.matmul(
                            out=pm[:], lhsT=xt_sb[:, b * P:(b + 1) * P],
                            rhs=w_sb[1][:], start=True, stop=True)
                    for j in range(nb):
                        b = b0 + j
                        w = ws_[j]
                        nc.vector.tensor_mul(
                            out=y_own[:, b * 8 * HID:b * 8 * HID + w],
                            in0=pms[j][:, :w],
                            in1=dis_exp[:, b * 8 * HID:b * 8 * HID + w])
                a = y_own[:]
                src3 = bass.AP(a.tensor, a.offset,
                               [[a.ap[0][0], P], [HID, TILES], [1, HID]])
                dst3 = bass.AP(ag_in[:].tensor, 0,
                               [[HID, P], [P * HID, TILES], [1, HID]])
                nc.sync.dma_start(dst3, src3)

            # wrapped-col offset of each range's token block
            g_woff = [0]
            for r in range(4):
                g_woff.append(g_woff[-1] + n_instr_r[r] * MAXCOL * 8)

            def message_pass_gr():
                for r, ii, t0, T, K in instrs:
                    g = gp.tile([P, MAXCOL * HID], F32, tag="g", name="g")
                    out3 = bass.AP(g[:].tensor, g[:].offset,
                                   [[g[:].ap[0][0], P], [HID, MAXCOL], [1, HID]])
                    gather(idx_g, g_woff[r] + ii * MAXCOL * 8, MAXCOL * P, out3,
                           WIN * r, WIN)
                    k = K
                    while k > 1:
                        h = (k + 1) // 2
                        srcn = k - h
                        nc.vector.tensor_add(
                            out=g[:, :srcn * T * HID],
                            in0=g[:, :srcn * T * HID],
                            in1=g[:, h * T * HID:(h + srcn) * T * HID])
                        k = h

            def message_pass_gonly():
                for r, ii, t0, T, K in instrs:
                    g = gp.tile([P, MAXCOL * HID], F32, tag="g", name="g")
                    out3 = bass.AP(g[:].tensor, g[:].offset,
                                   [[g[:].ap[0][0], P], [HID, MAXCOL], [1, HID]])
                    gather(idx_g, g_woff[r] + ii * MAXCOL * 8, MAXCOL * P, out3,
                           WIN * r, WIN)

            def message_pass():
                """gathers + reduces into slab[0..3]; slabs 1-3 via rslab."""
                BATCH = 5
                for bi in range(0, len(instrs), BATCH):
                    batch = instrs[bi:bi + BATCH]
                    gs = []
                    for r, ii, t0, T, K in batch:
                        g = gp.tile([P, MAXCOL * HID], F32, tag="g", name="g")
                        gs.append(g)
                        out3 = bass.AP(g[:].tensor, g[:].offset,
                                       [[g[:].ap[0][0], P], [HID, MAXCOL], [1, HID]])
                        gather(idx_g, g_woff[r] + ii * MAXCOL * 8, MAXCOL * P,
                               out3, WIN * r, WIN)
                    for g, (r, ii, t0, T, K) in zip(gs, batch):
                        k = K
                        while k > 1:
                            h = (k + 1) // 2
                            srcn = k - h
                            nc.vector.tensor_add(
                                out=g[:, :srcn * T * HID],
                                in0=g[:, :srcn * T * HID],
                                in1=g[:, h * T * HID:(h + srcn) * T * HID])
                            k = h
                        nc.vector.tensor_copy(
                            out=slab[r][:, t0 * HID:(t0 + T) * HID],
                            in_=g[:, :T * HID])
                # slabs 1-3 -> rslab -> align gather back into slab[r]
                for r in range(1, 4):
                    sl = slab[r][:]
                    src3 = bass.AP(sl.tensor, sl.offset,
                                   [[sl.ap[0][0], P], [HID, TILES], [1, HID]])
                    dst3 = bass.AP(rslab[:].tensor, (r - 1) * NPAD * ROW,
                                   [[ROW, P], [P * ROW, TILES], [1, HID]])
                    nc.sync.dma_start(dst3, src3)
                for r in range(1, 4):
                    awoff = (r - 1) * (P * 8)
                    done = 0
                    while done < TILES:
                        ntl = min(MAXCOL, TILES - done)
                        n_idx = ntl * P
                        ga = gp.tile([P, MAXCOL * HID], F32, tag="g", name="g")
                        out3 = bass.AP(ga[:].tensor, ga[:].offset,
                                       [[ga[:].ap[0][0], P], [HID, ntl], [1, HID]])
                        in_ap = bass.AP(rslab[:].tensor, (r - 1) * NPAD * ROW,
                                        [[ROW, NPAD], [1, HID]])
                        _dma_gather(
                            nc.gpsimd, out_ap=out3, in_ap=in_ap,
                            idxs_ap=idx_a[:, awoff + done * 8:
                                          awoff + (done + ntl) * 8],
                            num_idxs=n_idx, num_idxs_reg=n_idx,
                            elem_size=HID, elem_step=ROW,
                            single_packet=False, queue_num=qn[0] % 4)
                        qn[0] += 1
                        sl = slab[r][:]
                        dst_ap = bass.AP(sl.tensor, sl.offset + done * HID,
                                         [[sl.ap[0][0], P], [HID, ntl], [1, HID]])
                        nc.vector.tensor_copy(out=dst_ap, in_=out3)
                        done += ntl

            def pooling_and_head():
                first = True
                woff = 0
                for r, coff, K, acc in pool_chunks:
                    n_idx = K * P
                    g = gp.tile([P, MAXCOL * HID], F32, tag="g")
                    out3 = bass.AP(g[:].tensor, g[:].offset,
                                   [[g[:].ap[0][0], P], [HID, K], [1, HID]])
                    gather(idx_p, woff, n_idx, out3, WIN * r, WIN)
                    woff += MAXCOL * 8
                    k = K
                    while k > 1:
                        h = (k + 1) // 2
                        srcn = k - h
                        nc.vector.tensor_add(
                            out=g[:, :srcn * HID], in0=g[:, :srcn * HID],
                            in1=g[:, h * HID:(h + srcn) * HID])
                        k = h
                    if first:
                        nc.vector.tensor_copy(out=pool_slab[:], in_=g[:, :HID])
                        first = False
                    else:
                        nc.vector.tensor_add(out=pool_slab[:], in0=pool_slab[:],
                                             in1=g[:, :HID])
                # mean
                rcp = gp.tile([P, 1], F32, tag="rcp")
                nc.vector.reciprocal(out=rcp[:], in_=cnt_sb[:])
                nc.vector.tensor_scalar(out=pool_slab[:], in0=pool_slab[:],
                                        scalar1=rcp[:], scalar2=None,
                                        op0=AL.mult)

                def rrelu(ap):
                    pos = gp.tile([P, HID], F32, tag="rr1")
                    nc.vector.tensor_scalar(out=pos[:, :ap.shape[1]], in0=ap,
                                            scalar1=0.0, scalar2=None, op0=AL.max)
                    nc.vector.tensor_scalar(out=ap, in0=ap, scalar1=0.0,
                                            scalar2=RRELU_SLOPE, op0=AL.min,
                                            op1=AL.mult)
                    nc.vector.tensor_add(out=ap, in0=ap,
                                         in1=pos[:, :ap.shape[1]])

                # lin1
                pt = pst.tile([P, P], F32, tag="tp", space="PSUM")
                nc.tensor.transpose(out=pt[:HID, :], in_=pool_slab[:],
                                    identity=ident[:])
                gt = gp.tile([HID, P], F32, tag="gt")
                nc.vector.tensor_copy(out=gt[:], in_=pt[:HID, :])
                pm = pst.tile([P, HID], F32, tag="tp", space="PSUM", name="pmp")
                nc.tensor.matmul(out=pm[:], lhsT=gt[:], rhs=l1w_sb[:],
                                 start=True, stop=True)
                g1 = gp.tile([P, HID], F32, tag="g1")
                nc.vector.tensor_add(out=g1[:], in0=pm[:], in1=l1b_sb[:])
                rrelu(g1[:])
                # lin2
                pt2 = pst.tile([P, P], F32, tag="tp", space="PSUM")
                nc.tensor.transpose(out=pt2[:HID, :], in_=g1[:],
                                    identity=ident[:])
                gt2 = gp.tile([HID, P], F32, tag="gt")
                nc.vector.tensor_copy(out=gt2[:], in_=pt2[:HID, :])
                pm2 = pst.tile([P, 1], F32, tag="tp", space="PSUM", name="pmp2")
                nc.tensor.matmul(out=pm2[:], lhsT=gt2[:], rhs=l2w_sb[:],
                                 start=True, stop=True)
                g2 = gp.tile([P, 1], F32, tag="g2")
                nc.vector.tensor_add(out=g2[:], in0=pm2[:], in1=l2b_sb[:])
                rrelu(g2[:])
                nc.sync.dma_start(out_t[:], g2[:])

            if mode == "full":
                for rep in range(reps):
                    sc = (lambda name: nc.named_scope(f"r{rep}_{name}")) if reps > 1 else nc.named_scope
                    with sc("l1"):
                        layer1_y()
                    with sc("ag0"):
                        allgather_to_table()
                    for layer in range(1, 6):
                        with sc(f"mp{layer}"):
                            message_pass()
                        with sc(f"ep{layer}"):
                            epilogue_and_y(layer)
                        if layer < 5:
                            with sc(f"ag{layer}"):
                                allgather_to_table()
                    # h5 -> table
                    with sc("ag5"):
                        allgather_to_table()
                    with sc("pool"):
                        pooling_and_head()
            elif mode == "gr":
                layer1_y()
                allgather_to_table()
                for _ in range(reps):
                    for _l in range(5):
                        message_pass_gr()
                pooling_and_head()
            elif mode == "gonly":
                layer1_y()
                allgather_to_table()
                for _ in range(reps):
                    for _l in range(5):
                        message_pass_gonly()
                pooling_and_head()
            elif mode == "gathers":
                layer1_y()
                allgather_to_table()
                for _ in range(reps):
                    for _l in range(5):
                        message_pass()
                pooling_and_head()
            elif mode == "ag":
                layer1_y()
                for _ in range(reps):
                    for _l in range(6):
                        allgather_to_table()
                pooling_and_head()
            elif mode == "epilogue":
                layer1_y()
                allgather_to_table()
                message_pass()
                for _ in range(reps):
                    for layer in range(1, 6):
                        epilogue_and_y(layer)
                pooling_and_head()

    nc.finalize()
    return nc


def _make_in_maps(per_core, inputs):
    W1, W2, W3, W4, W5 = (inputs[f"W{i}"] for i in range(1, 6))
    bs = np.concatenate([np.asarray(inputs[f"b{i}"], np.float32)
                         for i in range(1, 6)]).reshape(1, 5 * HID)
    bs = np.repeat(bs, P, axis=0).copy()
    l1b = np.repeat(np.asarray(inputs["lin1_b"], np.float32).reshape(1, HID), P, 0).copy()
    l2b = np.repeat(np.asarray(inputs["lin2_b"], np.float32).reshape(1, 1), P, 0).copy()

    in_maps = []
    for c in range(C):
        pc = per_core[c]
        in_maps.append({
            "deg_tiles": pc["deg_tiles"].astype(np.float32),
            "xt": pc["xt"],
            "gather_w": pc["gather_w"],
            "align_w": pc["align_w"],
            "pool_w": pc["pool_w"],
            "cnt": pc["cnt"],
            "W1": np.kron(np.eye(8, dtype=np.float32), np.asarray(W1, np.float32)),
            "W2": np.kron(np.eye(8, dtype=np.float32), np.asarray(W2, np.float32)),
            "W3": np.kron(np.eye(8, dtype=np.float32), np.asarray(W3, np.float32)),
            "W4": np.kron(np.eye(8, dtype=np.float32), np.asarray(W4, np.float32)),
            "W5": np.kron(np.eye(8, dtype=np.float32), np.asarray(W5, np.float32)),
            "bs": bs,
            "lin1_w": np.asarray(inputs["lin1_w"], np.float32),
            "lin1_b": l1b,
            "lin2_w": np.asarray(inputs["lin2_w"], np.float32),
            "lin2_b": l2b,
        })
    return in_maps


def kernel(x, edge_index, batch, W1, b1, W2, b2, W3, b3, W4, b4, W5, b5,
           lin1_w, lin1_b, lin2_w, lin2_b, _reps=1, _prebuilt=None):
    per_core, plan = _preprocess(x, edge_index, batch)
    nc = _prebuilt if _prebuilt is not None else _build_program(plan, reps=_reps)
    inputs = dict(x=x, edge_index=edge_index, batch=batch, W1=W1, b1=b1, W2=W2,
                  b2=b2, W3=W3, b3=b3, W4=W4, b4=b4, W5=W5, b5=b5,
                  lin1_w=lin1_w, lin1_b=lin1_b, lin2_w=lin2_w, lin2_b=lin2_b)
    in_maps = _make_in_maps(per_core, inputs)

    res = run_bass_via_pjrt(nc, in_maps, n_cores=C)
    out = np.zeros((N_GRAPHS, 1), dtype=np.float32)
    for c in range(C):
        out[c * GPC:(c + 1) * GPC, 0] = res[c]["out"][:GPC, 0]
    return out

